# revision 1
# baseline (speedup 1.0000x reference)
"""Trainium2 Bass kernel for nn_Decoder_arch2 (LSTM image-caption decoder).

Reference computation (B=128, T=24 used steps, E=512, H2=1024, V=30000):
  tok = emb[captions]; seq = [pad_emb, tok[:, :23]]           # [B, 24, E]
  x_t = concat(seq_t, features)                               # [B, 2E]
  xg = x @ W_ih.T + b_ih + b_hh                               # [B, 24, 4096]
  24x LSTMCell steps (h = o*tanh(c), c = f*c + i*tanh(g))
  logits_t = h_t @ W_out.T + b_out                            # [B, 24, V]
  out = transpose(logits, (0, 2, 1))                          # [B, V, 24]
(The reference computes 25 steps and drops the last logit column, so step 25
and the last caption token are never needed.)

Sharding: pure data-parallel over batch. 8 cores x 16 batch rows each; every
core holds the full weights and computes its shard end-to-end. No collectives.

Device layouts (per core, partition dim always 128):
  gathered embeddings -> PE-transposed to xT[ec] [128(e), 384(t*16+b)] bf16
  xg_sb  [128, 24t, 32gc, 16b] fp32   (gate g = gc*128 + p)
  hs_sb  [128, 8hc, 24t, 16b] bf16    (hidden u = hc*128 + p)
  LSTM gates accumulate in one PSUM bank as [128, 32gc, 16b]
  projection: W_out tiles stationary, hs chunks moving, out [128(v), 384(t,b)]

Host pre-transposes/casts all weights (free layout prep) and reassembles the
[128, 30000, 24] output from the per-core [235, 128, 384] tensors.
"""

import sys

if "/opt/trn_rl_repo" not in sys.path:
    sys.path.insert(0, "/opt/trn_rl_repo")

import numpy as np
import ml_dtypes

import concourse.bass as bass
import concourse.bacc as bacc
import concourse.mybir as mybir
import concourse.tile as tile
from concourse.bass_utils import run_bass_kernel_spmd
from concourse.masks import make_identity

bf16 = ml_dtypes.bfloat16
F32 = mybir.dt.float32
BF16 = mybir.dt.bfloat16
I32 = mybir.dt.int32

B, T, E, V, H2 = 128, 24, 512, 30000, 1024
G = 4 * H2  # 4096
NC_N = 8
BS = B // NC_N  # 16 batch rows per core
NVT = 235  # ceil(30000/128)
VP = NVT * 128  # 30080
NCOL = T * BS  # 384 moving columns (t*16 + b)
STG = 4  # vt tiles per output staging DMA


def build_nc():
    nc = bacc.Bacc(None, target_bir_lowering=False)

    emb_d = nc.dram_tensor("embB", [V, E], BF16, kind="ExternalInput")
    idx_d = nc.dram_tensor("idx", [128, 3], I32, kind="ExternalInput")
    feat_d = nc.dram_tensor("featT", [128, 4, BS], BF16, kind="ExternalInput")
    wih_d = nc.dram_tensor("wihT", [8, 128, G], BF16, kind="ExternalInput")
    whh_d = nc.dram_tensor("whhT", [8, 128, G], BF16, kind="ExternalInput")
    bsum_d = nc.dram_tensor("bsum", [128, 32, BS], F32, kind="ExternalInput")
    bout_d = nc.dram_tensor("bout", [128, NVT], F32, kind="ExternalInput")
    wop_d = nc.dram_tensor("wop", [NVT, 128, H2], BF16, kind="ExternalInput")
    out_d = nc.dram_tensor("out", [NVT, 128, NCOL], F32, kind="ExternalOutput")

    with tile.TileContext(nc) as tc:
        with (
            tc.tile_pool(name="const", bufs=1) as const,
            tc.tile_pool(name="ge", bufs=3) as gep,
            tc.tile_pool(name="xt", bufs=4) as xtp,
            tc.tile_pool(name="w", bufs=8) as wp,
            tc.tile_pool(name="big", bufs=1) as big,
            tc.tile_pool(name="tmp", bufs=2) as tmp,
            tc.tile_pool(name="wout", bufs=12) as woutp,
            tc.tile_pool(name="stage", bufs=3) as stagep,
            tc.tile_pool(name="pm", bufs=2, space="PSUM") as pmp,
            tc.tile_pool(name="pf", bufs=1, space="PSUM") as pfp,
            tc.tile_pool(name="pg", bufs=2, space="PSUM") as pgp,
            tc.tile_pool(name="po", bufs=3, space="PSUM") as pop,
        ):
            # ---- constants / small inputs ----
            idx_sb = const.tile([128, 3], I32)
            nc.sync.dma_start(idx_sb[:], idx_d[:])
            feat_sb = const.tile([128, 4, BS], BF16)
            nc.sync.dma_start(feat_sb[:], feat_d[:])
            bsum_sb = const.tile([128, 32, BS], F32)
            nc.sync.dma_start(bsum_sb[:], bsum_d[:])
            bout_sb = const.tile([128, NVT], F32)
            nc.sync.dma_start(bout_sb[:], bout_d[:])
            ident = const.tile([128, 128], BF16)
            make_identity(nc, ident)

            # ---- W_ih tiles (4 seq + 4 feat), then W_hh reuses the slots ----
            wih_seq = []
            for ec in range(4):
                t_ = wp.tile([128, G], BF16, tag="w")
                nc.sync.dma_start(t_[:], wih_d[ec])
                wih_seq.append(t_)
            wih_feat = []
            for ec in range(4):
                t_ = wp.tile([128, G], BF16, tag="w")
                nc.sync.dma_start(t_[:], wih_d[4 + ec])
                wih_feat.append(t_)

            # ---- embedding gather + transpose into xT ----
            ge = []
            for r in range(3):
                g_t = gep.tile([128, E], BF16)
                nc.gpsimd.indirect_dma_start(
                    out=g_t[:],
                    out_offset=None,
                    in_=emb_d[:],
                    in_offset=bass.IndirectOffsetOnAxis(ap=idx_sb[:, r : r + 1], axis=0),
                )
                ge.append(g_t)

            xt = [xtp.tile([128, NCOL], BF16, tag="xt", name=f"xt{i}") for i in range(4)]
            for ec in range(4):
                for r in range(3):
                    pt = pmp.tile([128, 128], BF16, tag="pm")
                    nc.tensor.transpose(
                        pt[:], ge[r][:, ec * 128 : (ec + 1) * 128], ident[:]
                    )
                    nc.vector.tensor_copy(
                        xt[ec][:, r * 128 : (r + 1) * 128], pt[:]
                    )

            # ---- feature-side gate projection fg = W_ih[:, E:] @ feat + bsum ----
            psum_fg = pfp.tile([128, 32, BS], F32)
            for gc in range(32):
                for ec in range(4):
                    nc.tensor.matmul(
                        psum_fg[:, gc, :],
                        wih_feat[ec][:, gc * 128 : (gc + 1) * 128],
                        feat_sb[:, ec, :],
                        start=(ec == 0),
                        stop=(ec == 3),
                    )
            fg_sb = big.tile([128, 32, BS], F32, tag="fg")
            nc.vector.tensor_add(fg_sb[:], psum_fg[:], bsum_sb[:])

            # ---- xg GEMM (token side): xg[g, (t,b)] = W_ih[:, :E] @ seq ----
            xg_sb = big.tile([128, T, 32, BS], F32, tag="xg")
            for gc in range(32):
                psum_xg = pmp.tile([128, T, BS], F32, tag="pm")
                for ec in range(4):
                    nc.tensor.matmul(
                        psum_xg[:],
                        wih_seq[ec][:, gc * 128 : (gc + 1) * 128],
                        xt[ec][:],
                        start=(ec == 0),
                        stop=(ec == 3),
                    )
                nc.scalar.activation(
                    xg_sb[:, :, gc, :], psum_xg[:], mybir.ActivationFunctionType.Copy
                )
            # add fg (and bias) for every timestep
            for t in range(T):
                nc.vector.tensor_add(xg_sb[:, t], xg_sb[:, t], fg_sb[:])

            # ---- W_hh tiles (reuse the 8 "w" slots) ----
            whh = []
            for hc in range(8):
                t_ = wp.tile([128, G], BF16, tag="w")
                nc.sync.dma_start(t_[:], whh_d[hc])
                whh.append(t_)

            # ---- LSTM ----
            hs_sb = big.tile([128, 8, T, BS], BF16, tag="hs")
            c_sb = big.tile([128, 8, BS], F32, tag="c")
            SIG = mybir.ActivationFunctionType.Sigmoid
            TANH = mybir.ActivationFunctionType.Tanh

            for t in range(T):
                if t == 0:
                    gsrc = xg_sb[:, 0]  # [128, 32, BS], c=0, h=0
                else:
                    pg_t = pgp.tile([128, 32, BS], F32, tag="pg")
                    for gc in range(32):
                        for hc in range(8):
                            nc.tensor.matmul(
                                pg_t[:, gc, :],
                                whh[hc][:, gc * 128 : (gc + 1) * 128],
                                hs_sb[:, hc, t - 1, :],
                                start=(hc == 0),
                                stop=(hc == 7),
                            )
                    nc.vector.tensor_add(pg_t[:], pg_t[:], xg_sb[:, t])
                    gsrc = pg_t

                t_i = tmp.tile([128, 8, BS], F32, tag="ti")
                t_f = tmp.tile([128, 8, BS], F32, tag="tf", name="t_f") if t > 0 else None
                t_g = tmp.tile([128, 8, BS], F32, tag="tg")
                t_o = tmp.tile([128, 8, BS], F32, tag="to")
                t_c = tmp.tile([128, 8, BS], F32, tag="tc")
                nc.scalar.activation(t_i[:], gsrc[:, 0:8, :], SIG)
                nc.scalar.activation(t_g[:], gsrc[:, 16:24, :], TANH)
                nc.scalar.activation(t_o[:], gsrc[:, 24:32, :], SIG)
                if t == 0:
                    nc.vector.tensor_mul(c_sb[:], t_i[:], t_g[:])
                else:
                    nc.scalar.activation(t_f[:], gsrc[:, 8:16, :], SIG)
                    nc.vector.tensor_mul(t_f[:], t_f[:], c_sb[:])
                    nc.vector.tensor_mul(t_i[:], t_i[:], t_g[:])
                    nc.vector.tensor_add(c_sb[:], t_f[:], t_i[:])
                nc.scalar.activation(t_c[:], c_sb[:], TANH)
                nc.vector.tensor_mul(hs_sb[:, :, t, :], t_o[:], t_c[:])

            # ---- output projection ----
            stage_t = None
            for vt in range(NVT):
                w_t = woutp.tile([128, H2], BF16, tag="wo")
                nc.sync.dma_start(w_t[:], wop_d[vt])
                po_t = pop.tile([128, T, BS], F32, tag="po")
                for hc in range(8):
                    nc.tensor.matmul(
                        po_t[:],
                        w_t[:, hc * 128 : (hc + 1) * 128],
                        hs_sb[:, hc],
                        start=(hc == 0),
                        stop=(hc == 7),
                    )
                sj = vt % STG
                if sj == 0:
                    stage_t = stagep.tile([128, STG, T, BS], F32, tag="st")
                nc.scalar.activation(
                    stage_t[:, sj],
                    po_t[:],
                    mybir.ActivationFunctionType.Identity,
                    bias=bout_sb[:, vt : vt + 1],
                )
                if sj == STG - 1 or vt == NVT - 1:
                    nv = sj + 1
                    dst = out_d[vt - sj : vt + 1].rearrange("j p c -> p j c")
                    src = stage_t[:, :nv].rearrange("p j t b -> p j (t b)")
                    nc.sync.dma_start(dst, src)

    nc.compile()
    return nc


def prep_host(features, captions, pad_idx, emb, W_ih, W_hh, b_ih, b_hh, W_out, b_out):
    """Host-side layout prep. Returns (shared dict, per-core list of dicts)."""
    from einops import rearrange

    features = np.asarray(features, dtype=np.float32)
    captions = np.asarray(captions).astype(np.int64)
    pad_idx = int(np.asarray(pad_idx))
    emb = np.asarray(emb, dtype=np.float32)
    W_ih = np.asarray(W_ih, dtype=np.float32)
    W_hh = np.asarray(W_hh, dtype=np.float32)
    b_ih = np.asarray(b_ih, dtype=np.float32)
    b_hh = np.asarray(b_hh, dtype=np.float32)
    W_out = np.asarray(W_out, dtype=np.float32)
    b_out = np.asarray(b_out, dtype=np.float32)

    embB = np.ascontiguousarray(emb.astype(bf16))
    wihT = np.ascontiguousarray(rearrange(W_ih, "g (kc p) -> kc p g", p=128).astype(bf16))
    whhT = np.ascontiguousarray(rearrange(W_hh, "g (hc p) -> hc p g", p=128).astype(bf16))
    bsum = rearrange(b_ih + b_hh, "(gc p) -> p gc", p=128).astype(np.float32)
    bsum = np.ascontiguousarray(np.repeat(bsum[:, :, None], BS, axis=2))

    W_out_p = np.zeros((VP, H2), np.float32)
    W_out_p[:V] = W_out
    wop = np.ascontiguousarray(
        rearrange(W_out_p, "(vt f) (hc p) -> vt p (hc f)", f=128, p=128).astype(bf16)
    )
    b_out_p = np.zeros((VP,), np.float32)
    b_out_p[:V] = b_out
    bout = np.ascontiguousarray(rearrange(b_out_p, "(vt p) -> p vt", p=128))

    shared = {"embB": embB, "wihT": wihT, "whhT": whhT, "bsum": bsum,
              "wop": wop, "bout": bout}

    per_core = []
    for c in range(NC_N):
        bsl = slice(c * BS, (c + 1) * BS)
        gidx = np.zeros((T, BS), np.int64)  # row r = t*BS + b
        gidx[0, :] = pad_idx
        gidx[1:, :] = captions[bsl, : T - 1].T
        idx = np.ascontiguousarray(
            gidx.reshape(3, 128).T.astype(np.int32)
        )  # [128, 3]: idx[p, r3] = gidx_flat[r3*128 + p]
        featT = np.ascontiguousarray(
            rearrange(features[bsl], "b (ec p) -> p ec b", p=128).astype(bf16)
        )
        per_core.append({"idx": idx, "featT": featT})
    return shared, per_core


_NC_CACHE = None


def kernel(**inputs) -> np.ndarray:
    global _NC_CACHE
    if _NC_CACHE is None:
        _NC_CACHE = build_nc()
    nc = _NC_CACHE

    shared, per_core = prep_host(**inputs)
    in_maps = [dict(shared, **pc) for pc in per_core]
    res = run_bass_kernel_spmd(nc, in_maps, core_ids=list(range(NC_N)))

    out = np.empty((B, V, T), np.float32)
    for c in range(NC_N):
        o = res.results[c]["out"]  # [NVT, 128, NCOL], col = t*BS + b
        o = o.reshape(NVT, 128, T, BS)
        o = o.transpose(3, 0, 1, 2).reshape(BS, VP, T)
        out[c * BS : (c + 1) * BS] = o[:, :V, :]
    return out



# revision 2
# speedup vs baseline: 52468.3158x; 52468.3158x over previous
"""Trainium2 Bass kernel for nn_Decoder_arch2 (LSTM image-caption decoder).

Reference computation (B=128, T=24 used steps, E=512, H2=1024, V=30000):
  tok = emb[captions]; seq = [pad_emb, tok[:, :23]]           # [B, 24, E]
  x_t = concat(seq_t, features)                               # [B, 2E]
  xg = x @ W_ih.T + b_ih + b_hh                               # [B, 24, 4096]
  24x LSTMCell steps (h = o*tanh(c), c = f*c + i*tanh(g))
  logits_t = h_t @ W_out.T + b_out                            # [B, 24, V]
  out = transpose(logits, (0, 2, 1))                          # [B, V, 24]
(The reference computes 25 steps and drops the last logit column, so step 25
and the last caption token are never needed.)

Sharding: pure data-parallel over batch. 8 cores x 16 batch rows each; every
core holds the full weights and computes its shard end-to-end. No collectives.

Device layouts (per core, partition dim always 128):
  gathered embeddings -> PE-transposed to xT[ec] [128(e), 384(t*16+b)] bf16
  xg_sb  [128, 24t, 32gc, 16b] bf16   (gate g = gc*128 + p; includes feat+bias)
  hs_sb  [128, 8hc, 24t, 16b] bf16    (hidden u = hc*128 + p)
  LSTM gates accumulate per-gate in separate PSUM banks (order f,g,i,o) so the
  sigmoid/tanh chain of one gate overlaps the matmul burst of the next.
  projection: W_out tiles stationary, hs chunks moving, out [128(v), 384(t,b)]
  logits staged in bf16; host upcasts to f32 (tolerance 2e-2 >> bf16 eps).

Host pre-transposes/casts all weights (free layout prep) and reassembles the
[128, 30000, 24] output from the per-core [235, 128, 384] tensors.
"""

import sys

if "/opt/trn_rl_repo" not in sys.path:
    sys.path.insert(0, "/opt/trn_rl_repo")

import numpy as np
import ml_dtypes

import concourse.bass as bass
import concourse.bacc as bacc
import concourse.mybir as mybir
import concourse.tile as tile
from concourse.bass_utils import run_bass_kernel_spmd
from concourse.masks import make_identity

bf16 = ml_dtypes.bfloat16
F32 = mybir.dt.float32
BF16 = mybir.dt.bfloat16
I32 = mybir.dt.int32

B, T, E, V, H2 = 128, 24, 512, 30000, 1024
G = 4 * H2  # 4096
NC_N = 8
BS = B // NC_N  # 16 batch rows per core
NVT = 235  # ceil(30000/128)
VP = NVT * 128  # 30080
NCOL = T * BS  # 384 moving columns (t*16 + b)
STG = 4  # vt tiles per output staging DMA

# torch gate order i,f,g,o -> gc ranges; compute order f,g,i,o so the
# c-update chain (needs f,g,i) finishes under the o matmul burst.
GATE_I, GATE_F, GATE_G, GATE_O = range(4)


def build_nc():
    nc = bacc.Bacc(None, target_bir_lowering=False)

    emb_d = nc.dram_tensor("embB", [V, E], BF16, kind="ExternalInput")
    idx_d = nc.dram_tensor("idx", [128, 3], I32, kind="ExternalInput")
    feat_d = nc.dram_tensor("featT", [128, 4, BS], BF16, kind="ExternalInput")
    wih_d = nc.dram_tensor("wihT", [8, 128, G], BF16, kind="ExternalInput")
    whh_d = nc.dram_tensor("whhT", [8, 128, G], BF16, kind="ExternalInput")
    bsum_d = nc.dram_tensor("bsum", [128, 32, BS], F32, kind="ExternalInput")
    bout_d = nc.dram_tensor("bout", [128, NVT], F32, kind="ExternalInput")
    wop_d = nc.dram_tensor("wop", [NVT, 128, H2], BF16, kind="ExternalInput")
    out_d = nc.dram_tensor("out", [NVT, 128, NCOL], BF16, kind="ExternalOutput")

    with tile.TileContext(nc) as tc:
        with (
            tc.tile_pool(name="const", bufs=1) as const,
            tc.tile_pool(name="ge", bufs=3) as gep,
            tc.tile_pool(name="xt", bufs=4) as xtp,
            tc.tile_pool(name="w", bufs=8) as wp,
            tc.tile_pool(name="whhd", bufs=4) as whhdp,
            tc.tile_pool(name="big", bufs=1) as big,
            tc.tile_pool(name="tmp", bufs=2) as tmp,
            tc.tile_pool(name="wout", bufs=8) as woutp,
            tc.tile_pool(name="stage", bufs=3) as stagep,
            tc.tile_pool(name="pa", bufs=2, space="PSUM") as pap,
            tc.tile_pool(name="pg", bufs=4, space="PSUM") as pgp,
            tc.tile_pool(name="po", bufs=2, space="PSUM") as pop,
        ):
            # ---- constants / small inputs ----
            idx_sb = const.tile([128, 3], I32)
            nc.sync.dma_start(idx_sb[:], idx_d[:])
            feat_sb = const.tile([128, 4, BS], BF16)
            nc.sync.dma_start(feat_sb[:], feat_d[:])
            bsum_sb = const.tile([128, 32, BS], F32)
            nc.sync.dma_start(bsum_sb[:], bsum_d[:])
            bout_sb = const.tile([128, NVT], F32)
            nc.sync.dma_start(bout_sb[:], bout_d[:])
            ident = const.tile([128, 128], BF16)
            make_identity(nc, ident)

            # ---- W_ih tiles: feat first (their slots are recycled early
            # for half of W_hh), then seq ----
            wih_feat = []
            for ec in range(4):
                t_ = wp.tile([128, G], BF16, tag="w")
                nc.sync.dma_start(t_[:], wih_d[4 + ec])
                wih_feat.append(t_)
            wih_seq = []
            for ec in range(4):
                t_ = wp.tile([128, G], BF16, tag="w")
                nc.sync.dma_start(t_[:], wih_d[ec])
                wih_seq.append(t_)
            # dedicated W_hh slots: DMA queued immediately (no dependency)
            whh = [None] * 8
            for hc in range(4, 8):
                t_ = whhdp.tile([128, G], BF16, tag="whhd")
                nc.sync.dma_start(t_[:], whh_d[hc])
                whh[hc] = t_

            # ---- embedding gather + transpose into xT ----
            ge = []
            for r in range(3):
                g_t = gep.tile([128, E], BF16)
                nc.gpsimd.indirect_dma_start(
                    out=g_t[:],
                    out_offset=None,
                    in_=emb_d[:],
                    in_offset=bass.IndirectOffsetOnAxis(ap=idx_sb[:, r : r + 1], axis=0),
                )
                ge.append(g_t)

            xt = [xtp.tile([128, NCOL], BF16, tag="xt", name=f"xt{i}") for i in range(4)]
            for ec in range(4):
                for r in range(3):
                    pt = pap.tile([128, 128], BF16, tag="pa")
                    nc.tensor.transpose(
                        pt[:], ge[r][:, ec * 128 : (ec + 1) * 128], ident[:]
                    )
                    nc.vector.tensor_copy(
                        xt[ec][:, r * 128 : (r + 1) * 128], pt[:]
                    )

            # ---- feature-side gate projection fg = W_ih[:, E:] @ feat + bsum ----
            psum_fg = pap.tile([128, 32, BS], F32, tag="pa")
            for gc in range(32):
                for ec in range(4):
                    nc.tensor.matmul(
                        psum_fg[:, gc, :],
                        wih_feat[ec][:, gc * 128 : (gc + 1) * 128],
                        feat_sb[:, ec, :],
                        start=(ec == 0),
                        stop=(ec == 3),
                    )
            fg_sb = big.tile([128, 32, BS], BF16, tag="fg")
            nc.vector.tensor_add(fg_sb[:], psum_fg[:], bsum_sb[:])

            # remaining W_hh tiles recycle the W_ih-feat slots (free after fg)
            for hc in range(4):
                t_ = wp.tile([128, G], BF16, tag="w")
                nc.sync.dma_start(t_[:], whh_d[hc])
                whh[hc] = t_

            # ---- xg GEMM (token side): xg[g, (t,b)] = W_ih[:, :E] @ seq ----
            xg_sb = big.tile([128, T, 32, BS], BF16, tag="xg")
            for gc in range(32):
                psum_xg = pap.tile([128, T, BS], F32, tag="pa")
                for ec in range(4):
                    nc.tensor.matmul(
                        psum_xg[:],
                        wih_seq[ec][:, gc * 128 : (gc + 1) * 128],
                        xt[ec][:],
                        start=(ec == 0),
                        stop=(ec == 3),
                    )
                nc.scalar.activation(
                    xg_sb[:, :, gc, :], psum_xg[:], mybir.ActivationFunctionType.Copy
                )
            # add fg (feat-side + biases) for every timestep
            for t in range(T):
                nc.vector.tensor_add(xg_sb[:, t], xg_sb[:, t], fg_sb[:])

            # ---- LSTM ----
            hs_sb = big.tile([128, 8, T, BS], BF16, tag="hs")
            c_sb = big.tile([128, 8, BS], F32, tag="c")
            SIG = mybir.ActivationFunctionType.Sigmoid
            TANH = mybir.ActivationFunctionType.Tanh

            def gate_slice(g):
                return slice(8 * g, 8 * (g + 1))

            for t in range(T):
                if t == 0:
                    # h=0, c=0: gates come straight from xg
                    t_i = tmp.tile([128, 8, BS], F32, tag="ti")
                    t_g = tmp.tile([128, 8, BS], F32, tag="tg")
                    t_o = tmp.tile([128, 8, BS], F32, tag="to")
                    t_c = tmp.tile([128, 8, BS], F32, tag="tc")
                    nc.scalar.activation(t_i[:], xg_sb[:, 0, gate_slice(GATE_I)], SIG)
                    nc.scalar.activation(t_g[:], xg_sb[:, 0, gate_slice(GATE_G)], TANH)
                    nc.scalar.activation(t_o[:], xg_sb[:, 0, gate_slice(GATE_O)], SIG)
                    nc.vector.tensor_mul(c_sb[:], t_i[:], t_g[:])
                    nc.scalar.activation(t_c[:], c_sb[:], TANH)
                    nc.vector.tensor_mul(hs_sb[:, :, 0, :], t_o[:], t_c[:])
                    continue

                h_prev = [hs_sb[:, hc, t - 1, :] for hc in range(8)]

                def gate_mm(g):
                    pg_t = pgp.tile([128, 8, BS], F32, tag="pg", name=f"pg{g}_{t}")
                    base = 8 * g
                    for gcl in range(8):
                        gc = base + gcl
                        for hc in range(8):
                            nc.tensor.matmul(
                                pg_t[:, gcl, :],
                                whh[hc][:, gc * 128 : (gc + 1) * 128],
                                h_prev[hc],
                                start=(hc == 0),
                                stop=(hc == 7),
                            )
                    return pg_t

                # f first: f*c can run under the g/i bursts
                pg_f = gate_mm(GATE_F)
                t_f = tmp.tile([128, 8, BS], F32, tag="tf")
                nc.vector.tensor_add(t_f[:], pg_f[:], xg_sb[:, t, gate_slice(GATE_F)])
                nc.scalar.activation(t_f[:], t_f[:], SIG)
                nc.vector.tensor_mul(t_f[:], t_f[:], c_sb[:])

                pg_g = gate_mm(GATE_G)
                t_g = tmp.tile([128, 8, BS], F32, tag="tg")
                nc.vector.tensor_add(t_g[:], pg_g[:], xg_sb[:, t, gate_slice(GATE_G)])
                nc.scalar.activation(t_g[:], t_g[:], TANH)

                pg_i = gate_mm(GATE_I)
                t_i = tmp.tile([128, 8, BS], F32, tag="ti")
                nc.vector.tensor_add(t_i[:], pg_i[:], xg_sb[:, t, gate_slice(GATE_I)])
                nc.scalar.activation(t_i[:], t_i[:], SIG)
                nc.vector.tensor_mul(t_i[:], t_i[:], t_g[:])
                nc.vector.tensor_add(c_sb[:], t_f[:], t_i[:])
                t_c = tmp.tile([128, 8, BS], F32, tag="tc")
                nc.scalar.activation(t_c[:], c_sb[:], TANH)

                pg_o = gate_mm(GATE_O)
                t_o = tmp.tile([128, 8, BS], F32, tag="to")
                nc.vector.tensor_add(t_o[:], pg_o[:], xg_sb[:, t, gate_slice(GATE_O)])
                nc.scalar.activation(t_o[:], t_o[:], SIG)
                nc.vector.tensor_mul(hs_sb[:, :, t, :], t_o[:], t_c[:])

            # ---- output projection ----
            stage_t = None
            for vt in range(NVT):
                w_t = woutp.tile([128, H2], BF16, tag="wo")
                nc.sync.dma_start(w_t[:], wop_d[vt])
                po_t = pop.tile([128, T, BS], F32, tag="po")
                for hc in range(8):
                    nc.tensor.matmul(
                        po_t[:],
                        w_t[:, hc * 128 : (hc + 1) * 128],
                        hs_sb[:, hc],
                        start=(hc == 0),
                        stop=(hc == 7),
                    )
                sj = vt % STG
                if sj == 0:
                    stage_t = stagep.tile([128, STG, T, BS], BF16, tag="st")
                # bias-add + downcast on the (otherwise idle) vector engine
                nc.vector.tensor_scalar_add(
                    stage_t[:, sj], po_t[:], bout_sb[:, vt : vt + 1]
                )
                if sj == STG - 1 or vt == NVT - 1:
                    nv = sj + 1
                    dst = out_d[vt - sj : vt + 1].rearrange("j p c -> p j c")
                    src = stage_t[:, :nv].rearrange("p j t b -> p j (t b)")
                    nc.sync.dma_start(dst, src)

    nc.compile()
    return nc


def prep_host(features, captions, pad_idx, emb, W_ih, W_hh, b_ih, b_hh, W_out, b_out):
    """Host-side layout prep. Returns (shared dict, per-core list of dicts)."""
    from einops import rearrange

    features = np.asarray(features, dtype=np.float32)
    captions = np.asarray(captions).astype(np.int64)
    pad_idx = int(np.asarray(pad_idx))
    emb = np.asarray(emb, dtype=np.float32)
    W_ih = np.asarray(W_ih, dtype=np.float32)
    W_hh = np.asarray(W_hh, dtype=np.float32)
    b_ih = np.asarray(b_ih, dtype=np.float32)
    b_hh = np.asarray(b_hh, dtype=np.float32)
    W_out = np.asarray(W_out, dtype=np.float32)
    b_out = np.asarray(b_out, dtype=np.float32)

    embB = np.ascontiguousarray(emb.astype(bf16))
    wihT = np.ascontiguousarray(rearrange(W_ih, "g (kc p) -> kc p g", p=128).astype(bf16))
    whhT = np.ascontiguousarray(rearrange(W_hh, "g (hc p) -> hc p g", p=128).astype(bf16))
    bsum = rearrange(b_ih + b_hh, "(gc p) -> p gc", p=128).astype(np.float32)
    bsum = np.ascontiguousarray(np.repeat(bsum[:, :, None], BS, axis=2))

    W_out_p = np.zeros((VP, H2), np.float32)
    W_out_p[:V] = W_out
    wop = np.ascontiguousarray(
        rearrange(W_out_p, "(vt f) (hc p) -> vt p (hc f)", f=128, p=128).astype(bf16)
    )
    b_out_p = np.zeros((VP,), np.float32)
    b_out_p[:V] = b_out
    bout = np.ascontiguousarray(rearrange(b_out_p, "(vt p) -> p vt", p=128))

    shared = {"embB": embB, "wihT": wihT, "whhT": whhT, "bsum": bsum,
              "wop": wop, "bout": bout}

    per_core = []
    for c in range(NC_N):
        bsl = slice(c * BS, (c + 1) * BS)
        gidx = np.zeros((T, BS), np.int64)  # row r = t*BS + b
        gidx[0, :] = pad_idx
        gidx[1:, :] = captions[bsl, : T - 1].T
        idx = np.ascontiguousarray(
            gidx.reshape(3, 128).T.astype(np.int32)
        )  # [128, 3]: idx[p, r3] = gidx_flat[r3*128 + p]
        featT = np.ascontiguousarray(
            rearrange(features[bsl], "b (ec p) -> p ec b", p=128).astype(bf16)
        )
        per_core.append({"idx": idx, "featT": featT})
    return shared, per_core


_NC_CACHE = None


def kernel(**inputs) -> np.ndarray:
    global _NC_CACHE
    if _NC_CACHE is None:
        _NC_CACHE = build_nc()
    nc = _NC_CACHE

    shared, per_core = prep_host(**inputs)
    in_maps = [dict(shared, **pc) for pc in per_core]
    res = run_bass_kernel_spmd(nc, in_maps, core_ids=list(range(NC_N)))

    out = np.empty((B, V, T), np.float32)
    for c in range(NC_N):
        o = res.results[c]["out"]  # [NVT, 128, NCOL] bf16, col = t*BS + b
        o = np.asarray(o, dtype=np.float32).reshape(NVT, 128, T, BS)
        o = o.transpose(3, 0, 1, 2).reshape(BS, VP, T)
        out[c * BS : (c + 1) * BS] = o[:, :V, :]
    return out
